# revision 11
# baseline (speedup 1.0000x reference)
"""GCN encoder on 8 trn2 NeuronCores — full on-device implementation.

Strategy (per sharding_hint): edges partitioned by destination node across the
8 cores, so segment-sums stay core-local in PSUM. Node features live in DRAM
tables pre-scaled by dinv[src]; per 128-edge chunk a gpsimd dma_gather pulls
the source rows into SBUF partitions, a one-hot dst indicator built on the
vector engine turns the segment-sum into TensorE matmuls accumulating in PSUM,
and the 128x128 weights (replicated) + LayerNorm/ReLU run as a fused epilogue
per 128-node window. dinv[dst] is applied as a per-partition scale, so the
edge coefficient dinv[src]*dinv[dst] never materializes per edge. Self-loops
are appended as ordinary edges. Between layers the per-core h1*dinv shards are
AllGathered (two chunked collectives, fp16). Mean-pool partials [64,128] are
computed per core with one more matmul per window and combined on host.
"""
import sys

sys.path.insert(0, "/opt/trn_rl_repo")

import math

import numpy as np

import concourse.bacc as bacc
import concourse.bass as bass
import concourse.mybir as mybir
import concourse.tile as tile
from concourse.bass_utils import run_bass_kernel_spmd

f32 = mybir.dt.float32
fp16 = mybir.dt.float16
i16 = mybir.dt.int16
Alu = mybir.AluOpType
Act = mybir.ActivationFunctionType

D = 128
N_NODES = 50000
N_EDGES = 800000
N_GRAPHS = 64
N_CORES = 8
EPS = 1e-5
BK = 32    # msg-buffer chunk-column budget per gather batch
CAPC = 7   # max chunks per dma_gather call
DMA_SCRATCH = 16384  # SWDGE carveout


def _ceil(a, b):
    return -(-a // b)


class Plan:
    pass


def _wrap_idx(idx):
    """int16 index array (len % 128 == 0) -> [128, len//16] wrapped layout."""
    n = len(idx)
    w = idx.reshape(n // 16, 16).T.astype(np.int16)  # [16, n/16]
    return np.tile(w, (8, 1))  # replicated to 128 partitions


def _plan(src, dst, n_nodes, n_cores, n_graphs):
    """Host-side planning: windows, per-core slots, chunk layout, batches."""
    p = Plan()
    W = _ceil(n_nodes, 128)          # global dst windows
    WPC = _ceil(W, n_cores)          # windows (slots) per core
    NPC = WPC * 128                  # padded nodes per core
    NT = NPC * n_cores               # total table rows
    SLO = (WPC + 1) // 2             # slots in the "lo" half (allgather chunk 0)
    LO = SLO * 128                   # lo rows per core
    HI = NPC - LO
    TLO = n_cores * LO               # lo table rows (gather table A size)
    THI = NT - TLO
    assert TLO <= 32768 and THI <= 32768, (TLO, THI)
    p.W, p.WPC, p.NPC, p.NT, p.SLO, p.LO, p.HI, p.TLO = W, WPC, NPC, NT, SLO, LO, HI, TLO
    p.n_cores = n_cores
    p.n_nodes = n_nodes
    p.n_graphs = n_graphs

    deg = np.bincount(dst, minlength=n_nodes).astype(np.float64) + 1.0
    p.dinv = (1.0 / np.sqrt(deg)).astype(np.float32)

    # all edges incl. self loops: (src_node, dst_node)
    allsrc = np.concatenate([src, np.arange(n_nodes, dtype=np.int64)])
    alldst = np.concatenate([dst, np.arange(n_nodes, dtype=np.int64)])
    win = alldst // 128
    dstoff = (alldst % 128).astype(np.int32)

    # per-core slot assignment: core c owns windows [c*WPC, (c+1)*WPC)
    wcnt = np.bincount(win, minlength=n_cores * WPC)
    p.slotwins = np.zeros((n_cores, WPC), dtype=np.int64)  # slot -> global window
    p.slotpos = np.zeros(n_cores * WPC, dtype=np.int64)    # global window -> slot
    for c in range(n_cores):
        wins = np.arange(c * WPC, (c + 1) * WPC)
        order = np.argsort(-wcnt[wins], kind="stable")
        p.slotwins[c] = wins[order]
        p.slotpos[wins[order]] = np.arange(WPC)

    # layer-2 table row for each node v (slot-major shard layout + AG chunks)
    v = np.arange(n_nodes, dtype=np.int64)
    c2 = v // NPC
    s2 = p.slotpos[v // 128]  # slot within owning core (0..WPC-1)
    r = s2 * 128 + (v % 128)
    p.t2 = np.where(r < LO, c2 * LO + r, TLO + c2 * HI + (r - LO))

    # per-layer table rows for edge sources
    rows1 = allsrc  # layer 1: xp table indexed by original node id
    rows2 = p.t2[allsrc]

    p.layers = []
    for rows in (rows1, rows2):
        L = Plan()
        isB = rows >= TLO
        key = win * 2 + isB
        order = np.argsort(key, kind="stable")
        L.rows = rows[order]
        L.doff = dstoff[order]
        L.key = key[order]
        # boundaries for each (window, grp)
        L.bnd = np.searchsorted(L.key, np.arange(2 * n_cores * WPC + 1))
        # per (core, slot, grp) counts
        cntA = np.zeros((n_cores, WPC), dtype=np.int64)
        cntB = np.zeros((n_cores, WPC), dtype=np.int64)
        for c in range(n_cores):
            for s in range(WPC):
                w = p.slotwins[c][s]
                cntA[c, s] = L.bnd[2 * w + 1] - L.bnd[2 * w]
                cntB[c, s] = L.bnd[2 * w + 2] - L.bnd[2 * w + 1]
        L.CA = _ceil(cntA, 128).max(axis=0)  # [WPC] chunks for grp A per slot
        L.CB = _ceil(cntB, 128).max(axis=0)
        L.K = L.CA + L.CB
        # batches: greedy pack slots, force boundary at SLO
        batches = []
        cur = []
        cols = 0
        for s in range(WPC):
            k = int(L.K[s])
            if cur and (cols + k > BK or s == SLO):
                batches.append(cur)
                cur, cols = [], 0
            cur.append(s)
            cols += k
        if cur:
            batches.append(cur)
        L.batches = batches
        L.MSGC = max(sum(int(L.K[s]) for s in b) for b in batches)
        p.layers.append(L)
    return p


def _core_data(p, layer_i, c):
    """Build idx (wrapped int16) + d (fp16 [128, sumK]) arrays for one core."""
    L = p.layers[layer_i]
    WPC, TLO = p.WPC, p.TLO
    idx_blocks = []
    d_cols = []
    for b in L.batches:
        for grp in (0, 1):
            C = L.CA if grp == 0 else L.CB
            gidx = []
            for s in b:
                cap = int(C[s]) * 128
                if cap == 0:
                    continue
                w = p.slotwins[c][s]
                lo, hi = L.bnd[2 * w + grp], L.bnd[2 * w + grp + 1]
                rows = L.rows[lo:hi] - (TLO if grp else 0)
                doff = L.doff[lo:hi]
                n = hi - lo
                rows_p = np.zeros(cap, dtype=np.int64)
                rows_p[:n] = rows
                doff_p = np.full(cap, -1.0, dtype=np.float32)
                doff_p[:n] = doff
                gidx.append(rows_p)
                d_cols.append(doff_p.reshape(-1, 128).T)  # [128, C[s]]
            if gidx:
                idx_blocks.append(np.concatenate(gidx))
    idx = np.concatenate(idx_blocks) if idx_blocks else np.zeros(0, dtype=np.int64)
    assert idx.max(initial=0) < 32768 and idx.min(initial=0) >= 0
    d = (np.concatenate(d_cols, axis=1) if d_cols
         else np.zeros((128, 0), dtype=np.float32))
    return _wrap_idx(idx), np.ascontiguousarray(d, dtype=np.float16)


def _build_nc(p):
    """Build the SPMD Bass program from plan structure (counts only)."""
    nc = bacc.Bacc("TRN2", num_devices=p.n_cores,
                   dynamic_dma_scratch_size=DMA_SCRATCH)
    WPC, SLO = p.WPC, p.SLO
    L1, L2 = p.layers
    totK1 = int(L1.K.sum())
    totK2 = int(L2.K.sum())
    n_idx1 = sum(int(L1.CA[s] + L1.CB[s]) for s in range(WPC)) * 128
    n_idx2 = sum(int(L2.CA[s] + L2.CB[s]) for s in range(WPC)) * 128

    # inputs (per core)
    xp_lo = nc.dram_tensor("xp_lo", [p.TLO, D], fp16, kind="ExternalInput")
    xp_hi = nc.dram_tensor("xp_hi", [p.NT - p.TLO, D], fp16, kind="ExternalInput")
    idx1_d = nc.dram_tensor("idx1", [128, n_idx1 // 16], i16, kind="ExternalInput")
    idx2_d = nc.dram_tensor("idx2", [128, n_idx2 // 16], i16, kind="ExternalInput")
    d1_d = nc.dram_tensor("d1", [128, totK1], fp16, kind="ExternalInput")
    d2_d = nc.dram_tensor("d2", [128, totK2], fp16, kind="ExternalInput")
    NG = p.n_graphs
    pool_d = nc.dram_tensor("poolind", [128, WPC * NG], fp16, kind="ExternalInput")
    dinv_d = nc.dram_tensor("dinvs", [128, WPC], f32, kind="ExternalInput")
    w1_d = nc.dram_tensor("w1", [D, D], fp16, kind="ExternalInput")
    w2_d = nc.dram_tensor("w2", [D, D], fp16, kind="ExternalInput")
    gb_d = nc.dram_tensor("gammab", [128, D], f32, kind="ExternalInput")
    bb_d = nc.dram_tensor("betab", [128, D], f32, kind="ExternalInput")
    b1_d = nc.dram_tensor("b1cb", [128, D], f32, kind="ExternalInput")
    iota_d = nc.dram_tensor("iota", [128, 128], fp16, kind="ExternalInput")
    out_d = nc.dram_tensor("g_part", [NG, D], f32, kind="ExternalOutput")

    # internal DRAM
    h1own_lo = nc.dram_tensor("h1own_lo", [p.LO, D], fp16, kind="Internal")
    h1own_hi = nc.dram_tensor("h1own_hi", [p.HI, D], fp16, kind="Internal")
    h1t_lo = nc.dram_tensor("h1t_lo", [p.TLO, D], fp16, kind="Internal",
                            addr_space="Shared")
    h1t_hi = nc.dram_tensor("h1t_hi", [p.NT - p.TLO, D], fp16, kind="Internal",
                            addr_space="Shared")

    rg = [list(range(p.n_cores))]

    with tile.TileContext(nc) as tc:
        with (
            tc.tile_pool(name="const", bufs=1) as cpool,
            tc.tile_pool(name="msg", bufs=2) as mpool,
            tc.tile_pool(name="st", bufs=2) as spool,
            tc.tile_pool(name="work", bufs=3) as wpool,
            tc.tile_pool(name="psA", bufs=2, space="PSUM") as psA,
            tc.tile_pool(name="psZ", bufs=2, space="PSUM") as psZ,
            tc.tile_pool(name="psG", bufs=2, space="PSUM") as psG,
        ):
            def load_const(dram, shape, dtype, tag):
                t = cpool.tile(shape, dtype, tag=tag)
                nc.sync.dma_start(t[:], dram[:])
                return t

            idx1 = load_const(idx1_d, [128, n_idx1 // 16], i16, "idx1")
            idx2 = load_const(idx2_d, [128, n_idx2 // 16], i16, "idx2")
            d1 = load_const(d1_d, [128, totK1], fp16, "d1")
            d2 = load_const(d2_d, [128, totK2], fp16, "d2")
            poolind = load_const(pool_d, [128, WPC * NG], fp16, "poolind")
            dinvs = load_const(dinv_d, [128, WPC], f32, "dinvs")
            w1 = load_const(w1_d, [D, D], fp16, "w1")
            w2 = load_const(w2_d, [D, D], fp16, "w2")
            gb = load_const(gb_d, [128, D], f32, "gb")
            bb = load_const(bb_d, [128, D], f32, "bb")
            b1cb = load_const(b1_d, [128, D], f32, "b1cb")
            iota = load_const(iota_d, [128, 128], fp16, "iota")
            eps_col = cpool.tile([128, 1], f32, tag="eps")
            nc.vector.memset(eps_col[:], EPS)
            g_acc = cpool.tile([NG, D], f32, tag="g_acc")
            nc.vector.memset(g_acc[:], 0.0)

            def do_layer(layer_i, idx_sb, d_sb, tab_lo, tab_hi, w_sb):
                Lp = p.layers[layer_i]
                icol = 0  # running idx column (16 idx per col)
                dcol = 0  # running d/msg chunk column
                for b in Lp.batches:
                    nA = sum(int(Lp.CA[s]) for s in b)
                    nB = sum(int(Lp.CB[s]) for s in b)
                    kc = nA + nB
                    MC = Lp.MSGC
                    msg = mpool.tile([128, MC, 128], fp16, tag="msg")
                    st = spool.tile([128, MC, 128], fp16, tag="st")
                    for grp, n_ch, tab in ((0, nA, tab_lo), (1, nB, tab_hi)):
                        off = 0 if grp == 0 else nA
                        for k0 in range(0, n_ch, CAPC):
                            kn = min(CAPC, n_ch - k0)
                            ni = kn * 128
                            nc.gpsimd.dma_gather(
                                msg[:, off + k0:off + k0 + kn, :],
                                tab[:, :],
                                idx_sb[:, icol:icol + ni // 16],
                                ni, ni, D,
                            )
                            icol += ni // 16
                    # one-hot dst indicator for the whole batch
                    dsl = d_sb[:, dcol:dcol + kc]
                    nc.vector.tensor_tensor(
                        out=st[:, 0:kc, :],
                        in0=dsl.unsqueeze(2).broadcast_to([128, kc, 128]),
                        in1=iota[:].unsqueeze(1).broadcast_to([128, kc, 128]),
                        op=Alu.is_equal,
                    )
                    bcol = 0  # col offset within batch, A region then B region
                    aoff, boff = 0, nA
                    for s in b:
                        ka, kb = int(Lp.CA[s]), int(Lp.CB[s])
                        cols = list(range(aoff, aoff + ka)) + \
                               list(range(boff, boff + kb))
                        aoff += ka
                        boff += kb
                        kk = ka + kb
                        if kk == 0:
                            continue
                        agg = psA.tile([128, 128], f32, tag="agg")
                        for j, col in enumerate(cols):
                            nc.tensor.matmul(
                                agg[:],
                                lhsT=msg[:, col, :],
                                rhs=st[:, col, :],
                                start=(j == 0),
                                stop=(j == kk - 1),
                            )
                        aggs = wpool.tile([128, 128], fp16, tag="aggs")
                        nc.scalar.copy(aggs[:], agg[:])
                        zp = psZ.tile([128, 128], f32, tag="z")
                        dv = dinvs[:, s:s + 1]
                        if layer_i == 0:
                            nc.tensor.matmul(zp[:], lhsT=aggs[:], rhs=w_sb[:],
                                             start=True, stop=True)
                            # z*dinv[dst] + b1 -> LayerNorm -> ReLU, then
                            # *dinv again as the next layer's table prescale
                            zb = wpool.tile([128, 128], f32, tag="zb")
                            zsum = wpool.tile([128, 1], f32, tag="zsum")
                            nc.scalar.activation(zb[:], zp[:], Act.Identity,
                                                 scale=dv, accum_out=zsum[:])
                            mu = wpool.tile([128, 1], f32, tag="mu")
                            nc.scalar.mul(mu[:], zsum[:], 1.0 / D)
                            t0 = wpool.tile([128, 128], f32, tag="t0")
                            nc.vector.tensor_scalar_sub(t0[:], zb[:], mu[:])
                            t = wpool.tile([128, 128], f32, tag="t")
                            nc.vector.tensor_add(t[:], t0[:], b1cb[:])
                            tsq = wpool.tile([128, 128], f32, tag="tsq")
                            vsum = wpool.tile([128, 1], f32, tag="vsum")
                            nc.scalar.activation(tsq[:], t[:], Act.Square,
                                                 accum_out=vsum[:])
                            std = wpool.tile([128, 1], f32, tag="std")
                            nc.scalar.activation(std[:], vsum[:], Act.Sqrt,
                                                 bias=eps_col[:], scale=1.0 / D)
                            rstd = wpool.tile([128, 1], f32, tag="rstd")
                            nc.vector.reciprocal(rstd[:], std[:])
                            tg = wpool.tile([128, 128], f32, tag="tg")
                            nc.vector.tensor_scalar_mul(tg[:], t[:], rstd[:])
                            y = wpool.tile([128, 128], f32, tag="y")
                            nc.vector.tensor_mul(y[:], tg[:], gb[:])
                            y2 = wpool.tile([128, 128], f32, tag="y2")
                            nc.vector.tensor_add(y2[:], y[:], bb[:])
                            h1q = wpool.tile([128, 128], fp16, tag="h1q")
                            nc.scalar.activation(h1q[:], y2[:], Act.Relu,
                                                 scale=dv)
                            if s < SLO:
                                dst_ap = h1own_lo[s * 128:(s + 1) * 128, :]
                            else:
                                dst_ap = h1own_hi[(s - SLO) * 128:
                                                  (s - SLO + 1) * 128, :]
                            nc.sync.dma_start(dst_ap, h1q[:])
                            if s == SLO - 1:
                                nc.gpsimd.collective_compute(
                                    "AllGather", Alu.bypass, rg,
                                    [h1own_lo[:, :]], [h1t_lo[:, :]])
                            if s == WPC - 1:
                                nc.gpsimd.collective_compute(
                                    "AllGather", Alu.bypass, rg,
                                    [h1own_hi[:, :]], [h1t_hi[:, :]])
                        else:
                            nc.tensor.matmul(zp[:], lhsT=aggs[:], rhs=w_sb[:],
                                             start=True, stop=True)
                            z2q = wpool.tile([128, 128], fp16, tag="z2q")
                            nc.scalar.activation(z2q[:], zp[:], Act.Copy,
                                                 scale=dv)
                            gp = psG.tile([NG, 128], f32, tag="gp")
                            nc.tensor.matmul(
                                gp[:],
                                lhsT=poolind[:, s * NG:(s + 1) * NG],
                                rhs=z2q[:], start=True, stop=True)
                            nc.vector.tensor_add(g_acc[:], g_acc[:], gp[:])
                    dcol += kc

            do_layer(0, idx1, d1, xp_lo, xp_hi, w1)
            do_layer(1, idx2, d2, h1t_lo, h1t_hi, w2)
            nc.sync.dma_start(out_d[:, :], g_acc[:])

    nc.compile()
    return nc


def _prep_inputs(p, x, batch, W1, b1, gamma, beta, W2):
    """Host-side tensor prep shared across cores + per-core arrays."""
    n_nodes, WPC = p.n_nodes, p.WPC
    xp = np.zeros((p.NT, D), dtype=np.float16)
    xp[:n_nodes] = (x * p.dinv[:, None]).astype(np.float16)
    com = {
        "xp_lo": xp[:p.TLO],
        "xp_hi": xp[p.TLO:],
        "w1": np.ascontiguousarray(W1.astype(np.float16)),
        "w2": np.ascontiguousarray(W2.astype(np.float16)),
        "gammab": np.ascontiguousarray(
            np.broadcast_to(gamma.astype(np.float32), (128, D))),
        "betab": np.ascontiguousarray(
            np.broadcast_to(beta.astype(np.float32), (128, D))),
        "b1cb": np.ascontiguousarray(np.broadcast_to(
            (b1 - b1.mean()).astype(np.float32), (128, D))),
        "iota": np.ascontiguousarray(
            np.broadcast_to(np.arange(128, dtype=np.float16), (128, 128))),
    }
    in_maps = []
    for c in range(p.n_cores):
        idx1, d1 = _core_data(p, 0, c)
        idx2, d2 = _core_data(p, 1, c)
        # per-slot node -> batch(graph) pool indicator and dinv columns
        NG = p.n_graphs
        pool = np.zeros((128, WPC * NG), dtype=np.float16)
        dinvs = np.ones((128, WPC), dtype=np.float32)
        for s in range(WPC):
            w = int(p.slotwins[c][s])
            v0 = w * 128
            n = min(128, n_nodes - v0)
            if n <= 0:
                continue
            vs = np.arange(v0, v0 + n)
            pool[np.arange(n), s * NG + batch[vs]] = 1.0
            dinvs[:n, s] = p.dinv[vs]
        m = dict(com)
        m.update({"idx1": idx1, "idx2": idx2, "d1": d1, "d2": d2,
                  "poolind": pool, "dinvs": dinvs})
        in_maps.append(m)
    return in_maps


def run_gcn(x, src, dst, batch, W1, b1, gamma, beta, W2, b2,
            n_nodes, n_graphs, n_cores=N_CORES, trace=False):
    x = np.asarray(x, dtype=np.float32)
    src = np.asarray(src).astype(np.int64)
    dst = np.asarray(dst).astype(np.int64)
    batch = np.asarray(batch).astype(np.int64)
    p = _plan(src, dst, n_nodes, n_cores, n_graphs)
    nc = _build_nc(p)
    in_maps = _prep_inputs(p, x, batch, np.asarray(W1), np.asarray(b1),
                           np.asarray(gamma), np.asarray(beta), np.asarray(W2))
    res = run_bass_kernel_spmd(nc, in_maps, list(range(n_cores)), trace=trace)
    run_gcn.last_nc = nc
    run_gcn.last_plan = p
    gsum = np.sum([r["g_part"] for r in res.results], axis=0)
    counts = np.bincount(batch, minlength=n_graphs).astype(np.float32)
    g = gsum / np.maximum(counts, 1.0)[:, None] + np.asarray(b2, np.float32)
    # reference gives 0 + b2*0 contribution for empty graphs? No: reference
    # pools zeros then adds nothing (b2 enters per-node, pre-pool) -> 0.
    g[counts == 0] = 0.0
    return g.astype(np.float32), res


def kernel(x, src, dst, batch, W1, b1, gamma, beta, W2, b2):
    g, _ = run_gcn(x, src, dst, batch, W1, b1, gamma, beta, W2, b2,
                   N_NODES, N_GRAPHS)
    return g


# revision 14
# speedup vs baseline: 1.0997x; 1.0997x over previous
"""GCN encoder on 8 trn2 NeuronCores — full on-device implementation.

Strategy (per sharding_hint): edges partitioned by destination node across the
8 cores, so segment-sums stay core-local in PSUM. Node features live in DRAM
tables pre-scaled by dinv[src]; per 128-edge chunk a gpsimd dma_gather pulls
the source rows into SBUF partitions, a one-hot dst indicator built on the
vector engine turns the segment-sum into TensorE matmuls accumulating in PSUM,
and the 128x128 weights (replicated) + LayerNorm/ReLU run as a fused epilogue
per 128-node window. dinv[dst] is applied as a per-partition scale, so the
edge coefficient dinv[src]*dinv[dst] never materializes per edge. Self-loops
are appended as ordinary edges. Between layers the per-core h1*dinv shards are
AllGathered (two chunked collectives, fp16). Mean-pool partials [64,128] are
computed per core with one more matmul per window and combined on host.
"""
import sys

sys.path.insert(0, "/opt/trn_rl_repo")

import math

import numpy as np

import concourse.bacc as bacc
import concourse.bass as bass
import concourse.mybir as mybir
import concourse.tile as tile
from concourse.bass_utils import run_bass_kernel_spmd

f32 = mybir.dt.float32
fp16 = mybir.dt.float16
i16 = mybir.dt.int16
Alu = mybir.AluOpType
Act = mybir.ActivationFunctionType

D = 128
N_NODES = 50000
N_EDGES = 800000
N_GRAPHS = 64
N_CORES = 8
EPS = 1e-5
BK = 48    # msg-buffer chunk-column budget per gather batch
CAPC = 7   # max chunks per dma_gather call
DMA_SCRATCH = 16384  # SWDGE carveout (ring is runtime-fixed at 1024 descs)


def _ceil(a, b):
    return -(-a // b)


class Plan:
    pass


def _wrap_idx(idx):
    """int16 index array (len % 128 == 0) -> [128, len//16] wrapped layout."""
    n = len(idx)
    w = idx.reshape(n // 16, 16).T.astype(np.int16)  # [16, n/16]
    return np.tile(w, (8, 1))  # replicated to 128 partitions


def _plan(src, dst, n_nodes, n_cores, n_graphs):
    """Host-side planning: windows, per-core slots, chunk layout, batches."""
    p = Plan()
    W = _ceil(n_nodes, 128)          # global dst windows
    WPC = _ceil(W, n_cores)          # windows (slots) per core
    NPC = WPC * 128                  # padded nodes per core
    NT = NPC * n_cores               # total table rows
    SLO = (WPC + 1) // 2             # slots in the "lo" half (allgather chunk 0)
    LO = SLO * 128                   # lo rows per core
    HI = NPC - LO
    TLO = n_cores * LO               # lo table rows (gather table A size)
    THI = NT - TLO
    assert TLO <= 32768 and THI <= 32768, (TLO, THI)
    p.W, p.WPC, p.NPC, p.NT, p.SLO, p.LO, p.HI, p.TLO = W, WPC, NPC, NT, SLO, LO, HI, TLO
    p.n_cores = n_cores
    p.n_nodes = n_nodes
    p.n_graphs = n_graphs

    deg = np.bincount(dst, minlength=n_nodes).astype(np.float64) + 1.0
    p.dinv = (1.0 / np.sqrt(deg)).astype(np.float32)

    # all edges incl. self loops: (src_node, dst_node)
    allsrc = np.concatenate([src, np.arange(n_nodes, dtype=np.int64)])
    alldst = np.concatenate([dst, np.arange(n_nodes, dtype=np.int64)])
    win = alldst // 128
    dstoff = (alldst % 128).astype(np.int32)

    # snake-deal windows to cores by descending size: slot s holds windows of
    # near-identical size on every core, so the max-over-cores chunk padding
    # is tight.
    wcnt = np.bincount(win, minlength=n_cores * WPC)
    order = np.argsort(-wcnt, kind="stable")  # all windows, big first
    p.slotwins = np.zeros((n_cores, WPC), dtype=np.int64)  # slot -> global window
    p.slotpos = np.zeros(n_cores * WPC, dtype=np.int64)    # window -> slot
    p.corewin = np.zeros(n_cores * WPC, dtype=np.int64)    # window -> core
    for s in range(WPC):
        row = order[s * n_cores:(s + 1) * n_cores]
        if s % 2:
            row = row[::-1]
        for c in range(n_cores):
            w = row[c]
            p.slotwins[c, s] = w
            p.slotpos[w] = s
            p.corewin[w] = c

    # layer-2 table row for each node v (slot-major shard layout + AG chunks)
    v = np.arange(n_nodes, dtype=np.int64)
    c2 = p.corewin[v // 128]
    s2 = p.slotpos[v // 128]  # slot within owning core (0..WPC-1)
    r = s2 * 128 + (v % 128)
    p.t2 = np.where(r < LO, c2 * LO + r, TLO + c2 * HI + (r - LO))

    # per-layer table rows for edge sources
    rows1 = allsrc  # layer 1: xp table indexed by original node id
    rows2 = p.t2[allsrc]

    p.layers = []
    for rows in (rows1, rows2):
        L = Plan()
        isB = rows >= TLO
        key = win * 2 + isB
        order = np.argsort(key, kind="stable")
        L.rows = rows[order]
        L.doff = dstoff[order]
        L.key = key[order]
        # boundaries for each (window, grp)
        L.bnd = np.searchsorted(L.key, np.arange(2 * n_cores * WPC + 1))
        # per (core, slot, grp) counts
        cntA = np.zeros((n_cores, WPC), dtype=np.int64)
        cntB = np.zeros((n_cores, WPC), dtype=np.int64)
        for c in range(n_cores):
            for s in range(WPC):
                w = p.slotwins[c][s]
                cntA[c, s] = L.bnd[2 * w + 1] - L.bnd[2 * w]
                cntB[c, s] = L.bnd[2 * w + 2] - L.bnd[2 * w + 1]
        L.CA = _ceil(cntA, 128).max(axis=0)  # [WPC] chunks for grp A per slot
        L.CB = _ceil(cntB, 128).max(axis=0)
        L.K = L.CA + L.CB
        # batches: greedy pack slots, force boundary at SLO
        batches = []
        cur = []
        cols = 0
        for s in range(WPC):
            k = int(L.K[s])
            if cur and (cols + k > BK or s == SLO):
                batches.append(cur)
                cur, cols = [], 0
            cur.append(s)
            cols += k
        if cur:
            batches.append(cur)
        L.batches = batches
        L.MSGC = max(sum(int(L.K[s]) for s in b) for b in batches)
        p.layers.append(L)
    return p


def _core_data(p, layer_i, c):
    """Build idx (wrapped int16) + d (fp16 [128, sumK]) arrays for one core."""
    L = p.layers[layer_i]
    WPC, TLO = p.WPC, p.TLO
    idx_blocks = []
    d_cols = []
    for b in L.batches:
        for grp in (0, 1):
            C = L.CA if grp == 0 else L.CB
            gidx = []
            for s in b:
                cap = int(C[s]) * 128
                if cap == 0:
                    continue
                w = p.slotwins[c][s]
                lo, hi = L.bnd[2 * w + grp], L.bnd[2 * w + grp + 1]
                rows = L.rows[lo:hi] - (TLO if grp else 0)
                doff = L.doff[lo:hi]
                n = hi - lo
                rows_p = np.zeros(cap, dtype=np.int64)
                rows_p[:n] = rows
                doff_p = np.full(cap, -1.0, dtype=np.float32)
                doff_p[:n] = doff
                gidx.append(rows_p)
                d_cols.append(doff_p.reshape(-1, 128).T)  # [128, C[s]]
            if gidx:
                idx_blocks.append(np.concatenate(gidx))
    idx = np.concatenate(idx_blocks) if idx_blocks else np.zeros(0, dtype=np.int64)
    assert idx.max(initial=0) < 32768 and idx.min(initial=0) >= 0
    d = (np.concatenate(d_cols, axis=1) if d_cols
         else np.zeros((128, 0), dtype=np.float32))
    return _wrap_idx(idx), np.ascontiguousarray(d, dtype=np.float16)


def _build_nc(p):
    """Build the SPMD Bass program from plan structure (counts only)."""
    nc = bacc.Bacc("TRN2", num_devices=p.n_cores,
                   dynamic_dma_scratch_size=DMA_SCRATCH,
                   num_swdge_queues=4)
    WPC, SLO = p.WPC, p.SLO
    L1, L2 = p.layers
    totK1 = int(L1.K.sum())
    totK2 = int(L2.K.sum())
    n_idx1 = sum(int(L1.CA[s] + L1.CB[s]) for s in range(WPC)) * 128
    n_idx2 = sum(int(L2.CA[s] + L2.CB[s]) for s in range(WPC)) * 128

    # inputs (per core)
    xp_lo = nc.dram_tensor("xp_lo", [p.TLO, D], fp16, kind="ExternalInput")
    xp_hi = nc.dram_tensor("xp_hi", [p.NT - p.TLO, D], fp16, kind="ExternalInput")
    idx1_d = nc.dram_tensor("idx1", [128, n_idx1 // 16], i16, kind="ExternalInput")
    idx2_d = nc.dram_tensor("idx2", [128, n_idx2 // 16], i16, kind="ExternalInput")
    d1_d = nc.dram_tensor("d1", [128, totK1], fp16, kind="ExternalInput")
    d2_d = nc.dram_tensor("d2", [128, totK2], fp16, kind="ExternalInput")
    NG = p.n_graphs
    pool_d = nc.dram_tensor("poolind", [128, WPC * NG], fp16, kind="ExternalInput")
    dinv_d = nc.dram_tensor("dinvs", [128, WPC], f32, kind="ExternalInput")
    w1_d = nc.dram_tensor("w1", [D, D], fp16, kind="ExternalInput")
    w2_d = nc.dram_tensor("w2", [D, D], fp16, kind="ExternalInput")
    gb_d = nc.dram_tensor("gammab", [128, D], f32, kind="ExternalInput")
    bb_d = nc.dram_tensor("betab", [128, D], f32, kind="ExternalInput")
    b1_d = nc.dram_tensor("b1cb", [128, D], f32, kind="ExternalInput")
    iota_d = nc.dram_tensor("iota", [128, 128], fp16, kind="ExternalInput")
    out_d = nc.dram_tensor("g_part", [NG, D], f32, kind="ExternalOutput")

    # internal DRAM
    h1own_lo = nc.dram_tensor("h1own_lo", [p.LO, D], fp16, kind="Internal")
    h1own_hi = nc.dram_tensor("h1own_hi", [p.HI, D], fp16, kind="Internal")
    h1t_lo = nc.dram_tensor("h1t_lo", [p.TLO, D], fp16, kind="Internal",
                            addr_space="Shared")
    h1t_hi = nc.dram_tensor("h1t_hi", [p.NT - p.TLO, D], fp16, kind="Internal",
                            addr_space="Shared")

    rg = [list(range(p.n_cores))]

    with tile.TileContext(nc) as tc:
        with (
            tc.tile_pool(name="const", bufs=1) as cpool,
            tc.tile_pool(name="msg", bufs=2) as mpool,
            tc.tile_pool(name="st", bufs=2) as spool,
            tc.tile_pool(name="work", bufs=3) as wpool,
            tc.tile_pool(name="psA", bufs=2, space="PSUM") as psA,
            tc.tile_pool(name="psZ", bufs=2, space="PSUM") as psZ,
            tc.tile_pool(name="psG", bufs=2, space="PSUM") as psG,
        ):
            def load_const(dram, shape, dtype, tag):
                t = cpool.tile(shape, dtype, tag=tag)
                nc.sync.dma_start(t[:], dram[:])
                return t

            idx1 = load_const(idx1_d, [128, n_idx1 // 16], i16, "idx1")
            idx2 = load_const(idx2_d, [128, n_idx2 // 16], i16, "idx2")
            d1 = load_const(d1_d, [128, totK1], fp16, "d1")
            d2 = load_const(d2_d, [128, totK2], fp16, "d2")
            poolind = load_const(pool_d, [128, WPC * NG], fp16, "poolind")
            dinvs = load_const(dinv_d, [128, WPC], f32, "dinvs")
            w1 = load_const(w1_d, [D, D], fp16, "w1")
            w2 = load_const(w2_d, [D, D], fp16, "w2")
            gb = load_const(gb_d, [128, D], f32, "gb")
            bb = load_const(bb_d, [128, D], f32, "bb")
            b1cb = load_const(b1_d, [128, D], f32, "b1cb")
            iota = load_const(iota_d, [128, 128], fp16, "iota")
            eps_col = cpool.tile([128, 1], f32, tag="eps")
            nc.vector.memset(eps_col[:], EPS)
            g_acc = cpool.tile([NG, D], f32, tag="g_acc")
            nc.vector.memset(g_acc[:], 0.0)

            def do_layer(layer_i, idx_sb, d_sb, tab_lo, tab_hi, w_sb):
                do_layer.q = getattr(do_layer, "q", 0)
                Lp = p.layers[layer_i]
                icol = 0  # running idx column (16 idx per col)
                dcol = 0  # running d/msg chunk column
                for b in Lp.batches:
                    nA = sum(int(Lp.CA[s]) for s in b)
                    nB = sum(int(Lp.CB[s]) for s in b)
                    kc = nA + nB
                    MC = Lp.MSGC
                    msg = mpool.tile([128, MC, 128], fp16, tag="msg")
                    st = spool.tile([128, MC, 128], fp16, tag="st")
                    for grp, n_ch, tab in ((0, nA, tab_lo), (1, nB, tab_hi)):
                        off = 0 if grp == 0 else nA
                        for k0 in range(0, n_ch, CAPC):
                            kn = min(CAPC, n_ch - k0)
                            ni = kn * 128
                            nc.gpsimd.dma_gather(
                                msg[:, off + k0:off + k0 + kn, :],
                                tab[:, :],
                                idx_sb[:, icol:icol + ni // 16],
                                ni, ni, D,
                                queue_num=do_layer.q % 4,
                            )
                            do_layer.q += 1
                            icol += ni // 16
                    # one-hot dst indicator for the whole batch
                    dsl = d_sb[:, dcol:dcol + kc]
                    nc.vector.tensor_tensor(
                        out=st[:, 0:kc, :],
                        in0=dsl.unsqueeze(2).broadcast_to([128, kc, 128]),
                        in1=iota[:].unsqueeze(1).broadcast_to([128, kc, 128]),
                        op=Alu.is_equal,
                    )
                    bcol = 0  # col offset within batch, A region then B region
                    aoff, boff = 0, nA
                    for s in b:
                        ka, kb = int(Lp.CA[s]), int(Lp.CB[s])
                        cols = list(range(aoff, aoff + ka)) + \
                               list(range(boff, boff + kb))
                        aoff += ka
                        boff += kb
                        kk = ka + kb
                        if kk == 0:
                            continue
                        agg = psA.tile([128, 128], f32, tag="agg")
                        for j, col in enumerate(cols):
                            nc.tensor.matmul(
                                agg[:],
                                lhsT=msg[:, col, :],
                                rhs=st[:, col, :],
                                start=(j == 0),
                                stop=(j == kk - 1),
                            )
                        aggs = wpool.tile([128, 128], fp16, tag="aggs")
                        nc.scalar.copy(aggs[:], agg[:])
                        zp = psZ.tile([128, 128], f32, tag="z")
                        dv = dinvs[:, s:s + 1]
                        if layer_i == 0:
                            nc.tensor.matmul(zp[:], lhsT=aggs[:], rhs=w_sb[:],
                                             start=True, stop=True)
                            # z*dinv[dst] + b1 -> LayerNorm -> ReLU, then
                            # *dinv again as the next layer's table prescale
                            zb = wpool.tile([128, 128], f32, tag="zb")
                            zsum = wpool.tile([128, 1], f32, tag="zsum")
                            nc.scalar.activation(zb[:], zp[:], Act.Identity,
                                                 scale=dv, accum_out=zsum[:])
                            mu = wpool.tile([128, 1], f32, tag="mu")
                            nc.scalar.mul(mu[:], zsum[:], 1.0 / D)
                            t0 = wpool.tile([128, 128], f32, tag="t0")
                            nc.vector.tensor_scalar_sub(t0[:], zb[:], mu[:])
                            t = wpool.tile([128, 128], f32, tag="t")
                            nc.vector.tensor_add(t[:], t0[:], b1cb[:])
                            tsq = wpool.tile([128, 128], f32, tag="tsq")
                            vsum = wpool.tile([128, 1], f32, tag="vsum")
                            nc.scalar.activation(tsq[:], t[:], Act.Square,
                                                 accum_out=vsum[:])
                            std = wpool.tile([128, 1], f32, tag="std")
                            nc.scalar.activation(std[:], vsum[:], Act.Sqrt,
                                                 bias=eps_col[:], scale=1.0 / D)
                            rstd = wpool.tile([128, 1], f32, tag="rstd")
                            nc.vector.reciprocal(rstd[:], std[:])
                            tg = wpool.tile([128, 128], f32, tag="tg")
                            nc.vector.tensor_scalar_mul(tg[:], t[:], rstd[:])
                            y = wpool.tile([128, 128], f32, tag="y")
                            nc.vector.tensor_mul(y[:], tg[:], gb[:])
                            y2 = wpool.tile([128, 128], f32, tag="y2")
                            nc.vector.tensor_add(y2[:], y[:], bb[:])
                            h1q = wpool.tile([128, 128], fp16, tag="h1q")
                            nc.scalar.activation(h1q[:], y2[:], Act.Relu,
                                                 scale=dv)
                            if s < SLO:
                                dst_ap = h1own_lo[s * 128:(s + 1) * 128, :]
                            else:
                                dst_ap = h1own_hi[(s - SLO) * 128:
                                                  (s - SLO + 1) * 128, :]
                            nc.sync.dma_start(dst_ap, h1q[:])
                            if s == SLO - 1:
                                nc.gpsimd.collective_compute(
                                    "AllGather", Alu.bypass, rg,
                                    [h1own_lo[:, :]], [h1t_lo[:, :]])
                            if s == WPC - 1:
                                nc.gpsimd.collective_compute(
                                    "AllGather", Alu.bypass, rg,
                                    [h1own_hi[:, :]], [h1t_hi[:, :]])
                        else:
                            nc.tensor.matmul(zp[:], lhsT=aggs[:], rhs=w_sb[:],
                                             start=True, stop=True)
                            z2q = wpool.tile([128, 128], fp16, tag="z2q")
                            nc.scalar.activation(z2q[:], zp[:], Act.Copy,
                                                 scale=dv)
                            gp = psG.tile([NG, 128], f32, tag="gp")
                            nc.tensor.matmul(
                                gp[:],
                                lhsT=poolind[:, s * NG:(s + 1) * NG],
                                rhs=z2q[:], start=True, stop=True)
                            nc.vector.tensor_add(g_acc[:], g_acc[:], gp[:])
                    dcol += kc

            do_layer(0, idx1, d1, xp_lo, xp_hi, w1)
            do_layer(1, idx2, d2, h1t_lo, h1t_hi, w2)
            nc.sync.dma_start(out_d[:, :], g_acc[:])

    nc.compile()
    return nc


def _prep_inputs(p, x, batch, W1, b1, gamma, beta, W2):
    """Host-side tensor prep shared across cores + per-core arrays."""
    n_nodes, WPC = p.n_nodes, p.WPC
    xp = np.zeros((p.NT, D), dtype=np.float16)
    xp[:n_nodes] = (x * p.dinv[:, None]).astype(np.float16)
    com = {
        "xp_lo": xp[:p.TLO],
        "xp_hi": xp[p.TLO:],
        "w1": np.ascontiguousarray(W1.astype(np.float16)),
        "w2": np.ascontiguousarray(W2.astype(np.float16)),
        "gammab": np.ascontiguousarray(
            np.broadcast_to(gamma.astype(np.float32), (128, D))),
        "betab": np.ascontiguousarray(
            np.broadcast_to(beta.astype(np.float32), (128, D))),
        "b1cb": np.ascontiguousarray(np.broadcast_to(
            (b1 - b1.mean()).astype(np.float32), (128, D))),
        "iota": np.ascontiguousarray(
            np.broadcast_to(np.arange(128, dtype=np.float16), (128, 128))),
    }
    in_maps = []
    for c in range(p.n_cores):
        idx1, d1 = _core_data(p, 0, c)
        idx2, d2 = _core_data(p, 1, c)
        # per-slot node -> batch(graph) pool indicator and dinv columns
        NG = p.n_graphs
        pool = np.zeros((128, WPC * NG), dtype=np.float16)
        dinvs = np.ones((128, WPC), dtype=np.float32)
        for s in range(WPC):
            w = int(p.slotwins[c][s])
            v0 = w * 128
            n = min(128, n_nodes - v0)
            if n <= 0:
                continue
            vs = np.arange(v0, v0 + n)
            pool[np.arange(n), s * NG + batch[vs]] = 1.0
            dinvs[:n, s] = p.dinv[vs]
        m = dict(com)
        m.update({"idx1": idx1, "idx2": idx2, "d1": d1, "d2": d2,
                  "poolind": pool, "dinvs": dinvs})
        in_maps.append(m)
    return in_maps


def run_gcn(x, src, dst, batch, W1, b1, gamma, beta, W2, b2,
            n_nodes, n_graphs, n_cores=N_CORES, trace=False):
    x = np.asarray(x, dtype=np.float32)
    src = np.asarray(src).astype(np.int64)
    dst = np.asarray(dst).astype(np.int64)
    batch = np.asarray(batch).astype(np.int64)
    p = _plan(src, dst, n_nodes, n_cores, n_graphs)
    nc = _build_nc(p)
    in_maps = _prep_inputs(p, x, batch, np.asarray(W1), np.asarray(b1),
                           np.asarray(gamma), np.asarray(beta), np.asarray(W2))
    res = run_bass_kernel_spmd(nc, in_maps, list(range(n_cores)), trace=trace)
    run_gcn.last_nc = nc
    run_gcn.last_plan = p
    gsum = np.sum([r["g_part"] for r in res.results], axis=0)
    counts = np.bincount(batch, minlength=n_graphs).astype(np.float32)
    g = gsum / np.maximum(counts, 1.0)[:, None] + np.asarray(b2, np.float32)
    # reference gives 0 + b2*0 contribution for empty graphs? No: reference
    # pools zeros then adds nothing (b2 enters per-node, pre-pool) -> 0.
    g[counts == 0] = 0.0
    return g.astype(np.float32), res


def kernel(x, src, dst, batch, W1, b1, gamma, beta, W2, b2):
    g, _ = run_gcn(x, src, dst, batch, W1, b1, gamma, beta, W2, b2,
                   N_NODES, N_GRAPHS)
    return g


# revision 15
# speedup vs baseline: 1.1028x; 1.0028x over previous
"""GCN encoder on 8 trn2 NeuronCores — full on-device implementation.

Strategy (per sharding_hint): edges partitioned by destination node across the
8 cores, so segment-sums stay core-local in PSUM. Node features live in DRAM
tables pre-scaled by dinv[src]; per 128-edge chunk a gpsimd dma_gather pulls
the source rows into SBUF partitions, a one-hot dst indicator built on the
vector engine turns the segment-sum into TensorE matmuls accumulating in PSUM,
and the 128x128 weights (replicated) + LayerNorm/ReLU run as a fused epilogue
per 128-node window. dinv[dst] is applied as a per-partition scale, so the
edge coefficient dinv[src]*dinv[dst] never materializes per edge. Self-loops
are appended as ordinary edges. Between layers the per-core h1*dinv shards are
AllGathered (two chunked collectives, fp16). Mean-pool partials [64,128] are
computed per core with one more matmul per window and combined on host.
"""
import sys

sys.path.insert(0, "/opt/trn_rl_repo")

import math

import numpy as np

import concourse.bacc as bacc
import concourse.bass as bass
import concourse.mybir as mybir
import concourse.tile as tile
from concourse.bass_utils import run_bass_kernel_spmd

f32 = mybir.dt.float32
fp16 = mybir.dt.float16
i16 = mybir.dt.int16
Alu = mybir.AluOpType
Act = mybir.ActivationFunctionType

D = 128
N_NODES = 50000
N_EDGES = 800000
N_GRAPHS = 64
N_CORES = 8
EPS = 1e-5
BK = 48    # msg-buffer chunk-column budget per gather batch
CAPC = 7   # max chunks per dma_gather call
DMA_SCRATCH = 16384  # SWDGE carveout (ring is runtime-fixed at 1024 descs)


def _ceil(a, b):
    return -(-a // b)


class Plan:
    pass


def _wrap_idx(idx):
    """int16 index array (len % 128 == 0) -> [128, len//16] wrapped layout."""
    n = len(idx)
    w = idx.reshape(n // 16, 16).T.astype(np.int16)  # [16, n/16]
    return np.tile(w, (8, 1))  # replicated to 128 partitions


def _plan(src, dst, n_nodes, n_cores, n_graphs):
    """Host-side planning: windows, per-core slots, chunk layout, batches."""
    p = Plan()
    W = _ceil(n_nodes, 128)          # global dst windows
    WPC = _ceil(W, n_cores)          # windows (slots) per core
    NPC = WPC * 128                  # padded nodes per core
    NT = NPC * n_cores               # total table rows
    SLO = (WPC + 1) // 2             # slots in the "lo" half (allgather chunk 0)
    LO = SLO * 128                   # lo rows per core
    HI = NPC - LO
    TLO = n_cores * LO               # lo table rows (gather table A size)
    THI = NT - TLO
    assert TLO <= 32768 and THI <= 32768, (TLO, THI)
    p.W, p.WPC, p.NPC, p.NT, p.SLO, p.LO, p.HI, p.TLO = W, WPC, NPC, NT, SLO, LO, HI, TLO
    p.n_cores = n_cores
    p.n_nodes = n_nodes
    p.n_graphs = n_graphs

    deg = np.bincount(dst, minlength=n_nodes).astype(np.float64) + 1.0
    p.dinv = (1.0 / np.sqrt(deg)).astype(np.float32)

    # all edges incl. self loops: (src_node, dst_node)
    allsrc = np.concatenate([src, np.arange(n_nodes, dtype=np.int64)])
    alldst = np.concatenate([dst, np.arange(n_nodes, dtype=np.int64)])
    win = alldst // 128
    dstoff = (alldst % 128).astype(np.int32)

    # snake-deal windows to cores by descending size: slot s holds windows of
    # near-identical size on every core, so the max-over-cores chunk padding
    # is tight.
    wcnt = np.bincount(win, minlength=n_cores * WPC)
    # ascending: slots 0..SLO-1 (the AG_lo half) are the smallest windows, so
    # the first AllGather's inputs are ready earliest in layer 1
    order = np.argsort(wcnt, kind="stable")
    p.slotwins = np.zeros((n_cores, WPC), dtype=np.int64)  # slot -> global window
    p.slotpos = np.zeros(n_cores * WPC, dtype=np.int64)    # window -> slot
    p.corewin = np.zeros(n_cores * WPC, dtype=np.int64)    # window -> core
    for s in range(WPC):
        row = order[s * n_cores:(s + 1) * n_cores]
        if s % 2:
            row = row[::-1]
        for c in range(n_cores):
            w = row[c]
            p.slotwins[c, s] = w
            p.slotpos[w] = s
            p.corewin[w] = c

    # layer-2 table row for each node v (slot-major shard layout + AG chunks)
    v = np.arange(n_nodes, dtype=np.int64)
    c2 = p.corewin[v // 128]
    s2 = p.slotpos[v // 128]  # slot within owning core (0..WPC-1)
    r = s2 * 128 + (v % 128)
    p.t2 = np.where(r < LO, c2 * LO + r, TLO + c2 * HI + (r - LO))

    # per-layer table rows for edge sources
    rows1 = allsrc  # layer 1: xp table indexed by original node id
    rows2 = p.t2[allsrc]

    p.layers = []
    for rows in (rows1, rows2):
        L = Plan()
        isB = rows >= TLO
        key = win * 2 + isB
        order = np.argsort(key, kind="stable")
        L.rows = rows[order]
        L.doff = dstoff[order]
        L.key = key[order]
        # boundaries for each (window, grp)
        L.bnd = np.searchsorted(L.key, np.arange(2 * n_cores * WPC + 1))
        # per (core, slot, grp) counts
        cntA = np.zeros((n_cores, WPC), dtype=np.int64)
        cntB = np.zeros((n_cores, WPC), dtype=np.int64)
        for c in range(n_cores):
            for s in range(WPC):
                w = p.slotwins[c][s]
                cntA[c, s] = L.bnd[2 * w + 1] - L.bnd[2 * w]
                cntB[c, s] = L.bnd[2 * w + 2] - L.bnd[2 * w + 1]
        L.CA = _ceil(cntA, 128).max(axis=0)  # [WPC] chunks for grp A per slot
        L.CB = _ceil(cntB, 128).max(axis=0)
        L.K = L.CA + L.CB
        # batches: greedy pack slots, force boundary at SLO
        batches = []
        cur = []
        cols = 0
        for s in range(WPC):
            k = int(L.K[s])
            if cur and (cols + k > BK or s == SLO):
                batches.append(cur)
                cur, cols = [], 0
            cur.append(s)
            cols += k
        if cur:
            batches.append(cur)
        L.batches = batches
        L.MSGC = max(sum(int(L.K[s]) for s in b) for b in batches)
        p.layers.append(L)
    return p


def _core_data(p, layer_i, c):
    """Build idx (wrapped int16) + d (fp16 [128, sumK]) arrays for one core."""
    L = p.layers[layer_i]
    WPC, TLO = p.WPC, p.TLO
    idx_blocks = []
    d_cols = []
    for b in L.batches:
        for grp in (0, 1):
            C = L.CA if grp == 0 else L.CB
            gidx = []
            for s in b:
                cap = int(C[s]) * 128
                if cap == 0:
                    continue
                w = p.slotwins[c][s]
                lo, hi = L.bnd[2 * w + grp], L.bnd[2 * w + grp + 1]
                rows = L.rows[lo:hi] - (TLO if grp else 0)
                doff = L.doff[lo:hi]
                n = hi - lo
                rows_p = np.zeros(cap, dtype=np.int64)
                rows_p[:n] = rows
                doff_p = np.full(cap, -1.0, dtype=np.float32)
                doff_p[:n] = doff
                gidx.append(rows_p)
                d_cols.append(doff_p.reshape(-1, 128).T)  # [128, C[s]]
            if gidx:
                idx_blocks.append(np.concatenate(gidx))
    idx = np.concatenate(idx_blocks) if idx_blocks else np.zeros(0, dtype=np.int64)
    assert idx.max(initial=0) < 32768 and idx.min(initial=0) >= 0
    d = (np.concatenate(d_cols, axis=1) if d_cols
         else np.zeros((128, 0), dtype=np.float32))
    return _wrap_idx(idx), np.ascontiguousarray(d, dtype=np.float16)


def _build_nc(p):
    """Build the SPMD Bass program from plan structure (counts only)."""
    nc = bacc.Bacc("TRN2", num_devices=p.n_cores,
                   dynamic_dma_scratch_size=DMA_SCRATCH,
                   num_swdge_queues=4)
    WPC, SLO = p.WPC, p.SLO
    L1, L2 = p.layers
    totK1 = int(L1.K.sum())
    totK2 = int(L2.K.sum())
    n_idx1 = sum(int(L1.CA[s] + L1.CB[s]) for s in range(WPC)) * 128
    n_idx2 = sum(int(L2.CA[s] + L2.CB[s]) for s in range(WPC)) * 128

    # inputs (per core)
    xp_lo = nc.dram_tensor("xp_lo", [p.TLO, D], fp16, kind="ExternalInput")
    xp_hi = nc.dram_tensor("xp_hi", [p.NT - p.TLO, D], fp16, kind="ExternalInput")
    idx1_d = nc.dram_tensor("idx1", [128, n_idx1 // 16], i16, kind="ExternalInput")
    idx2_d = nc.dram_tensor("idx2", [128, n_idx2 // 16], i16, kind="ExternalInput")
    d1_d = nc.dram_tensor("d1", [128, totK1], fp16, kind="ExternalInput")
    d2_d = nc.dram_tensor("d2", [128, totK2], fp16, kind="ExternalInput")
    NG = p.n_graphs
    pool_d = nc.dram_tensor("poolind", [128, WPC * NG], fp16, kind="ExternalInput")
    dinv_d = nc.dram_tensor("dinvs", [128, WPC], f32, kind="ExternalInput")
    w1_d = nc.dram_tensor("w1", [D, D], fp16, kind="ExternalInput")
    w2_d = nc.dram_tensor("w2", [D, D], fp16, kind="ExternalInput")
    gb_d = nc.dram_tensor("gammab", [128, D], f32, kind="ExternalInput")
    bb_d = nc.dram_tensor("betab", [128, D], f32, kind="ExternalInput")
    b1_d = nc.dram_tensor("b1cb", [128, D], f32, kind="ExternalInput")
    iota_d = nc.dram_tensor("iota", [128, 128], fp16, kind="ExternalInput")
    out_d = nc.dram_tensor("g_part", [NG, D], f32, kind="ExternalOutput")

    # internal DRAM
    h1own_lo = nc.dram_tensor("h1own_lo", [p.LO, D], fp16, kind="Internal")
    h1own_hi = nc.dram_tensor("h1own_hi", [p.HI, D], fp16, kind="Internal")
    h1t_lo = nc.dram_tensor("h1t_lo", [p.TLO, D], fp16, kind="Internal",
                            addr_space="Shared")
    h1t_hi = nc.dram_tensor("h1t_hi", [p.NT - p.TLO, D], fp16, kind="Internal",
                            addr_space="Shared")

    rg = [list(range(p.n_cores))]

    with tile.TileContext(nc) as tc:
        with (
            tc.tile_pool(name="const", bufs=1) as cpool,
            tc.tile_pool(name="msg", bufs=2) as mpool,
            tc.tile_pool(name="st", bufs=2) as spool,
            tc.tile_pool(name="work", bufs=3) as wpool,
            tc.tile_pool(name="psA", bufs=2, space="PSUM") as psA,
            tc.tile_pool(name="psZ", bufs=2, space="PSUM") as psZ,
            tc.tile_pool(name="psG", bufs=2, space="PSUM") as psG,
        ):
            def load_const(dram, shape, dtype, tag):
                t = cpool.tile(shape, dtype, tag=tag)
                nc.sync.dma_start(t[:], dram[:])
                return t

            idx1 = load_const(idx1_d, [128, n_idx1 // 16], i16, "idx1")
            idx2 = load_const(idx2_d, [128, n_idx2 // 16], i16, "idx2")
            d1 = load_const(d1_d, [128, totK1], fp16, "d1")
            d2 = load_const(d2_d, [128, totK2], fp16, "d2")
            poolind = load_const(pool_d, [128, WPC * NG], fp16, "poolind")
            dinvs = load_const(dinv_d, [128, WPC], f32, "dinvs")
            w1 = load_const(w1_d, [D, D], fp16, "w1")
            w2 = load_const(w2_d, [D, D], fp16, "w2")
            gb = load_const(gb_d, [128, D], f32, "gb")
            bb = load_const(bb_d, [128, D], f32, "bb")
            b1cb = load_const(b1_d, [128, D], f32, "b1cb")
            iota = load_const(iota_d, [128, 128], fp16, "iota")
            eps_col = cpool.tile([128, 1], f32, tag="eps")
            nc.vector.memset(eps_col[:], EPS)
            g_acc = cpool.tile([NG, D], f32, tag="g_acc")
            nc.vector.memset(g_acc[:], 0.0)

            def do_layer(layer_i, idx_sb, d_sb, tab_lo, tab_hi, w_sb):
                do_layer.q = getattr(do_layer, "q", 0)
                Lp = p.layers[layer_i]
                icol = 0  # running idx column (16 idx per col)
                dcol = 0  # running d/msg chunk column
                for b in Lp.batches:
                    nA = sum(int(Lp.CA[s]) for s in b)
                    nB = sum(int(Lp.CB[s]) for s in b)
                    kc = nA + nB
                    MC = Lp.MSGC
                    msg = mpool.tile([128, MC, 128], fp16, tag="msg")
                    st = spool.tile([128, MC, 128], fp16, tag="st")
                    for grp, n_ch, tab in ((0, nA, tab_lo), (1, nB, tab_hi)):
                        off = 0 if grp == 0 else nA
                        for k0 in range(0, n_ch, CAPC):
                            kn = min(CAPC, n_ch - k0)
                            ni = kn * 128
                            nc.gpsimd.dma_gather(
                                msg[:, off + k0:off + k0 + kn, :],
                                tab[:, :],
                                idx_sb[:, icol:icol + ni // 16],
                                ni, ni, D,
                                queue_num=do_layer.q % 4,
                            )
                            do_layer.q += 1
                            icol += ni // 16
                    # one-hot dst indicator for the whole batch
                    dsl = d_sb[:, dcol:dcol + kc]
                    nc.vector.tensor_tensor(
                        out=st[:, 0:kc, :],
                        in0=dsl.unsqueeze(2).broadcast_to([128, kc, 128]),
                        in1=iota[:].unsqueeze(1).broadcast_to([128, kc, 128]),
                        op=Alu.is_equal,
                    )
                    bcol = 0  # col offset within batch, A region then B region
                    aoff, boff = 0, nA
                    for s in b:
                        ka, kb = int(Lp.CA[s]), int(Lp.CB[s])
                        cols = list(range(aoff, aoff + ka)) + \
                               list(range(boff, boff + kb))
                        aoff += ka
                        boff += kb
                        kk = ka + kb
                        if kk == 0:
                            continue
                        agg = psA.tile([128, 128], f32, tag="agg")
                        for j, col in enumerate(cols):
                            nc.tensor.matmul(
                                agg[:],
                                lhsT=msg[:, col, :],
                                rhs=st[:, col, :],
                                start=(j == 0),
                                stop=(j == kk - 1),
                            )
                        aggs = wpool.tile([128, 128], fp16, tag="aggs")
                        nc.scalar.copy(aggs[:], agg[:])
                        zp = psZ.tile([128, 128], f32, tag="z")
                        dv = dinvs[:, s:s + 1]
                        if layer_i == 0:
                            nc.tensor.matmul(zp[:], lhsT=aggs[:], rhs=w_sb[:],
                                             start=True, stop=True)
                            # z*dinv[dst] + b1 -> LayerNorm -> ReLU, then
                            # *dinv again as the next layer's table prescale
                            zb = wpool.tile([128, 128], f32, tag="zb")
                            zsum = wpool.tile([128, 1], f32, tag="zsum")
                            nc.scalar.activation(zb[:], zp[:], Act.Identity,
                                                 scale=dv, accum_out=zsum[:])
                            mu = wpool.tile([128, 1], f32, tag="mu")
                            nc.scalar.mul(mu[:], zsum[:], 1.0 / D)
                            t0 = wpool.tile([128, 128], f32, tag="t0")
                            nc.vector.tensor_scalar_sub(t0[:], zb[:], mu[:])
                            t = wpool.tile([128, 128], f32, tag="t")
                            nc.vector.tensor_add(t[:], t0[:], b1cb[:])
                            tsq = wpool.tile([128, 128], f32, tag="tsq")
                            vsum = wpool.tile([128, 1], f32, tag="vsum")
                            nc.scalar.activation(tsq[:], t[:], Act.Square,
                                                 accum_out=vsum[:])
                            std = wpool.tile([128, 1], f32, tag="std")
                            nc.scalar.activation(std[:], vsum[:], Act.Sqrt,
                                                 bias=eps_col[:], scale=1.0 / D)
                            rstd = wpool.tile([128, 1], f32, tag="rstd")
                            nc.vector.reciprocal(rstd[:], std[:])
                            tg = wpool.tile([128, 128], f32, tag="tg")
                            nc.vector.tensor_scalar_mul(tg[:], t[:], rstd[:])
                            y = wpool.tile([128, 128], f32, tag="y")
                            nc.vector.tensor_mul(y[:], tg[:], gb[:])
                            y2 = wpool.tile([128, 128], f32, tag="y2")
                            nc.vector.tensor_add(y2[:], y[:], bb[:])
                            h1q = wpool.tile([128, 128], fp16, tag="h1q")
                            nc.scalar.activation(h1q[:], y2[:], Act.Relu,
                                                 scale=dv)
                            if s < SLO:
                                dst_ap = h1own_lo[s * 128:(s + 1) * 128, :]
                            else:
                                dst_ap = h1own_hi[(s - SLO) * 128:
                                                  (s - SLO + 1) * 128, :]
                            nc.sync.dma_start(dst_ap, h1q[:])
                            if s == SLO - 1:
                                nc.gpsimd.collective_compute(
                                    "AllGather", Alu.bypass, rg,
                                    [h1own_lo[:, :]], [h1t_lo[:, :]])
                            if s == WPC - 1:
                                nc.gpsimd.collective_compute(
                                    "AllGather", Alu.bypass, rg,
                                    [h1own_hi[:, :]], [h1t_hi[:, :]])
                        else:
                            nc.tensor.matmul(zp[:], lhsT=aggs[:], rhs=w_sb[:],
                                             start=True, stop=True)
                            z2q = wpool.tile([128, 128], fp16, tag="z2q")
                            nc.scalar.activation(z2q[:], zp[:], Act.Copy,
                                                 scale=dv)
                            gp = psG.tile([NG, 128], f32, tag="gp")
                            nc.tensor.matmul(
                                gp[:],
                                lhsT=poolind[:, s * NG:(s + 1) * NG],
                                rhs=z2q[:], start=True, stop=True)
                            nc.vector.tensor_add(g_acc[:], g_acc[:], gp[:])
                    dcol += kc

            do_layer(0, idx1, d1, xp_lo, xp_hi, w1)
            do_layer(1, idx2, d2, h1t_lo, h1t_hi, w2)
            nc.sync.dma_start(out_d[:, :], g_acc[:])

    nc.compile()
    return nc


def _prep_inputs(p, x, batch, W1, b1, gamma, beta, W2):
    """Host-side tensor prep shared across cores + per-core arrays."""
    n_nodes, WPC = p.n_nodes, p.WPC
    xp = np.zeros((p.NT, D), dtype=np.float16)
    xp[:n_nodes] = (x * p.dinv[:, None]).astype(np.float16)
    com = {
        "xp_lo": xp[:p.TLO],
        "xp_hi": xp[p.TLO:],
        "w1": np.ascontiguousarray(W1.astype(np.float16)),
        "w2": np.ascontiguousarray(W2.astype(np.float16)),
        "gammab": np.ascontiguousarray(
            np.broadcast_to(gamma.astype(np.float32), (128, D))),
        "betab": np.ascontiguousarray(
            np.broadcast_to(beta.astype(np.float32), (128, D))),
        "b1cb": np.ascontiguousarray(np.broadcast_to(
            (b1 - b1.mean()).astype(np.float32), (128, D))),
        "iota": np.ascontiguousarray(
            np.broadcast_to(np.arange(128, dtype=np.float16), (128, 128))),
    }
    in_maps = []
    for c in range(p.n_cores):
        idx1, d1 = _core_data(p, 0, c)
        idx2, d2 = _core_data(p, 1, c)
        # per-slot node -> batch(graph) pool indicator and dinv columns
        NG = p.n_graphs
        pool = np.zeros((128, WPC * NG), dtype=np.float16)
        dinvs = np.ones((128, WPC), dtype=np.float32)
        for s in range(WPC):
            w = int(p.slotwins[c][s])
            v0 = w * 128
            n = min(128, n_nodes - v0)
            if n <= 0:
                continue
            vs = np.arange(v0, v0 + n)
            pool[np.arange(n), s * NG + batch[vs]] = 1.0
            dinvs[:n, s] = p.dinv[vs]
        m = dict(com)
        m.update({"idx1": idx1, "idx2": idx2, "d1": d1, "d2": d2,
                  "poolind": pool, "dinvs": dinvs})
        in_maps.append(m)
    return in_maps


def run_gcn(x, src, dst, batch, W1, b1, gamma, beta, W2, b2,
            n_nodes, n_graphs, n_cores=N_CORES, trace=False):
    x = np.asarray(x, dtype=np.float32)
    src = np.asarray(src).astype(np.int64)
    dst = np.asarray(dst).astype(np.int64)
    batch = np.asarray(batch).astype(np.int64)
    p = _plan(src, dst, n_nodes, n_cores, n_graphs)
    nc = _build_nc(p)
    in_maps = _prep_inputs(p, x, batch, np.asarray(W1), np.asarray(b1),
                           np.asarray(gamma), np.asarray(beta), np.asarray(W2))
    res = run_bass_kernel_spmd(nc, in_maps, list(range(n_cores)), trace=trace)
    run_gcn.last_nc = nc
    run_gcn.last_plan = p
    gsum = np.sum([r["g_part"] for r in res.results], axis=0)
    counts = np.bincount(batch, minlength=n_graphs).astype(np.float32)
    g = gsum / np.maximum(counts, 1.0)[:, None] + np.asarray(b2, np.float32)
    # reference gives 0 + b2*0 contribution for empty graphs? No: reference
    # pools zeros then adds nothing (b2 enters per-node, pre-pool) -> 0.
    g[counts == 0] = 0.0
    return g.astype(np.float32), res


def kernel(x, src, dst, batch, W1, b1, gamma, beta, W2, b2):
    g, _ = run_gcn(x, src, dst, batch, W1, b1, gamma, beta, W2, b2,
                   N_NODES, N_GRAPHS)
    return g


# revision 20
# speedup vs baseline: 1.8861x; 1.7103x over previous
"""GCN encoder on 8 trn2 NeuronCores — full on-device implementation.

Layer 1 (per sharding_hint): edges partitioned by destination node across the
8 cores so segment-sums stay core-local in PSUM. Node features live in DRAM
tables pre-scaled by dinv[src]; per 128-edge chunk a gpsimd dma_gather pulls
the source rows into SBUF partitions, a one-hot dst indicator built on the
vector engine turns the segment-sum into TensorE matmuls accumulating in PSUM,
and the 128x128 weights (replicated) + LayerNorm/ReLU run as a fused epilogue
per 128-node window. dinv[dst] is applied as a per-partition scale, so the
edge coefficient dinv[src]*dinv[dst] never materializes per edge. Self-loops
are appended as ordinary edges.

Layer 2 exploits the linearity of mean-pooling to avoid any inter-core
exchange: edges are re-partitioned by SOURCE owner, each core gathers only
from its local h1*dinv shard and accumulates the pooled partial
R[feat, graph] = sum_e h1'[src_e] x onehot(graph(dst_e))*dinv[dst_e] in one
PSUM tile (chunks need no dst grouping at all), then g_part = R.T @ W2. The
8 per-core partials are summed on host and divided by graph counts.
"""
import sys

sys.path.insert(0, "/opt/trn_rl_repo")

import math

import numpy as np

import concourse.bacc as bacc
import concourse.bass as bass
import concourse.mybir as mybir
import concourse.tile as tile
from concourse.bass_utils import run_bass_kernel_spmd

f32 = mybir.dt.float32
fp16 = mybir.dt.float16
i16 = mybir.dt.int16
Alu = mybir.AluOpType
Act = mybir.ActivationFunctionType

D = 128
N_NODES = 50000
N_EDGES = 800000
N_GRAPHS = 64
N_CORES = 8
EPS = 1e-5
BK = 48    # msg-buffer chunk-column budget per gather batch
CAPC = 7   # max chunks per dma_gather call
DMA_SCRATCH = 16384  # SWDGE carveout (ring is runtime-fixed at 1024 descs)


def _ceil(a, b):
    return -(-a // b)


class Plan:
    pass


def _wrap_idx(idx):
    """int16 index array (len % 128 == 0) -> [128, len//16] wrapped layout."""
    n = len(idx)
    w = idx.reshape(n // 16, 16).T.astype(np.int16)  # [16, n/16]
    return np.tile(w, (8, 1))  # replicated to 128 partitions


def _plan(src, dst, batch, n_nodes, n_cores, n_graphs):
    """Host-side planning: windows, per-core slots, chunk layout, batches."""
    p = Plan()
    W = _ceil(n_nodes, 128)          # global dst windows
    WPC = _ceil(W, n_cores)          # windows (slots) per core
    NPC = WPC * 128                  # padded nodes per core
    NT = NPC * n_cores               # total table rows
    SLO = (WPC + 1) // 2             # slots in the "lo" half (allgather chunk 0)
    LO = SLO * 128                   # lo rows per core
    HI = NPC - LO
    TLO = n_cores * LO               # lo table rows (gather table A size)
    THI = NT - TLO
    assert TLO <= 32768 and THI <= 32768, (TLO, THI)
    p.W, p.WPC, p.NPC, p.NT, p.SLO, p.LO, p.HI, p.TLO = W, WPC, NPC, NT, SLO, LO, HI, TLO
    p.n_cores = n_cores
    p.n_nodes = n_nodes
    p.n_graphs = n_graphs
    p.batch = batch

    deg = np.bincount(dst, minlength=n_nodes).astype(np.float64) + 1.0
    p.dinv = (1.0 / np.sqrt(deg)).astype(np.float32)

    # all edges incl. self loops: (src_node, dst_node)
    allsrc = np.concatenate([src, np.arange(n_nodes, dtype=np.int64)])
    alldst = np.concatenate([dst, np.arange(n_nodes, dtype=np.int64)])
    win = alldst // 128
    dstoff = (alldst % 128).astype(np.int32)

    # snake-deal windows to cores by descending size: slot s holds windows of
    # near-identical size on every core, so the max-over-cores chunk padding
    # is tight.
    wcnt = np.bincount(win, minlength=n_cores * WPC)
    # ascending: slots 0..SLO-1 (the AG_lo half) are the smallest windows, so
    # the first AllGather's inputs are ready earliest in layer 1
    order = np.argsort(wcnt, kind="stable")
    p.slotwins = np.zeros((n_cores, WPC), dtype=np.int64)  # slot -> global window
    p.slotpos = np.zeros(n_cores * WPC, dtype=np.int64)    # window -> slot
    p.corewin = np.zeros(n_cores * WPC, dtype=np.int64)    # window -> core
    for s in range(WPC):
        row = order[s * n_cores:(s + 1) * n_cores]
        if s % 2:
            row = row[::-1]
        for c in range(n_cores):
            w = row[c]
            p.slotwins[c, s] = w
            p.slotpos[w] = s
            p.corewin[w] = c

    # layer 2 is src-sharded: each core processes edges whose SOURCE it owns,
    # gathering from its local h1own shard (slot-major rows) and accumulating
    # the pooled partial R[feat, graph] directly — no halo AllGather needed
    # (pooling is linear, partials are summed on host).
    v = np.arange(n_nodes, dtype=np.int64)
    p.srcown = p.corewin[v // 128]                       # node -> owning core
    p.locrow = p.slotpos[v // 128] * 128 + (v % 128)     # node -> h1own row

    L2 = Plan()
    ecore = p.srcown[allsrc]
    eslot = p.slotpos[allsrc // 128]
    cnts = np.bincount(ecore, minlength=n_cores)
    L2.C2 = int(_ceil(int(cnts.max()), 128))             # chunks per core
    # per-core edge order: by src slot (earliest-written h1own rows first)
    L2.order = np.lexsort((eslot, ecore))
    L2.cbnd = np.searchsorted(ecore[L2.order], np.arange(n_cores + 1))
    L2.rows = p.locrow[allsrc[L2.order]]
    L2.gid = batch[alldst[L2.order]].astype(np.float32)
    L2.dvd = p.dinv[alldst[L2.order]].astype(np.float32)
    # batches: chunk ranges of <= BK columns
    L2.batches = [(k0, min(k0 + BK, L2.C2)) for k0 in range(0, L2.C2, BK)]
    p.layer2 = L2

    p.layers = []
    for rows in (allsrc,):
        L = Plan()
        isB = rows >= TLO
        key = win * 2 + isB
        order = np.argsort(key, kind="stable")
        L.rows = rows[order]
        L.doff = dstoff[order]
        L.key = key[order]
        # boundaries for each (window, grp)
        L.bnd = np.searchsorted(L.key, np.arange(2 * n_cores * WPC + 1))
        # per (core, slot, grp) counts
        cntA = np.zeros((n_cores, WPC), dtype=np.int64)
        cntB = np.zeros((n_cores, WPC), dtype=np.int64)
        for c in range(n_cores):
            for s in range(WPC):
                w = p.slotwins[c][s]
                cntA[c, s] = L.bnd[2 * w + 1] - L.bnd[2 * w]
                cntB[c, s] = L.bnd[2 * w + 2] - L.bnd[2 * w + 1]
        L.CA = _ceil(cntA, 128).max(axis=0)  # [WPC] chunks for grp A per slot
        L.CB = _ceil(cntB, 128).max(axis=0)
        L.K = L.CA + L.CB
        # batches: greedy pack slots, force boundary at SLO
        batches = []
        cur = []
        cols = 0
        for s in range(WPC):
            k = int(L.K[s])
            if cur and (cols + k > BK or s == SLO):
                batches.append(cur)
                cur, cols = [], 0
            cur.append(s)
            cols += k
        if cur:
            batches.append(cur)
        L.batches = batches
        L.MSGC = max(sum(int(L.K[s]) for s in b) for b in batches)
        p.layers.append(L)
    return p


def _core_data2(p, c):
    """Layer-2 per-core arrays: idx (wrapped), gid/dvd (fp16 [128, C2])."""
    L2 = p.layer2
    cap = L2.C2 * 128
    lo, hi = L2.cbnd[c], L2.cbnd[c + 1]
    n = hi - lo
    rows_p = np.zeros(cap, dtype=np.int64)
    rows_p[:n] = L2.rows[lo:hi]
    gid_p = np.full(cap, -1.0, dtype=np.float32)
    gid_p[:n] = L2.gid[lo:hi]
    dvd_p = np.ones(cap, dtype=np.float32)
    dvd_p[:n] = L2.dvd[lo:hi]
    assert rows_p.max(initial=0) < 32768
    return (_wrap_idx(rows_p),
            np.ascontiguousarray(gid_p.reshape(-1, 128).T, dtype=np.float16),
            np.ascontiguousarray(dvd_p.reshape(-1, 128).T, dtype=np.float16))


def _core_data(p, layer_i, c):
    """Build idx (wrapped int16) + d (fp16 [128, sumK]) arrays for one core."""
    L = p.layers[layer_i]
    WPC, TLO = p.WPC, p.TLO
    idx_blocks = []
    d_cols = []
    for b in L.batches:
        for grp in (0, 1):
            C = L.CA if grp == 0 else L.CB
            gidx = []
            for s in b:
                cap = int(C[s]) * 128
                if cap == 0:
                    continue
                w = p.slotwins[c][s]
                lo, hi = L.bnd[2 * w + grp], L.bnd[2 * w + grp + 1]
                rows = L.rows[lo:hi] - (TLO if grp else 0)
                doff = L.doff[lo:hi]
                n = hi - lo
                rows_p = np.zeros(cap, dtype=np.int64)
                rows_p[:n] = rows
                doff_p = np.full(cap, -1.0, dtype=np.float32)
                doff_p[:n] = doff
                gidx.append(rows_p)
                d_cols.append(doff_p.reshape(-1, 128).T)  # [128, C[s]]
            if gidx:
                idx_blocks.append(np.concatenate(gidx))
    idx = np.concatenate(idx_blocks) if idx_blocks else np.zeros(0, dtype=np.int64)
    assert idx.max(initial=0) < 32768 and idx.min(initial=0) >= 0
    d = (np.concatenate(d_cols, axis=1) if d_cols
         else np.zeros((128, 0), dtype=np.float32))
    return _wrap_idx(idx), np.ascontiguousarray(d, dtype=np.float16)


def _build_nc(p):
    """Build the SPMD Bass program from plan structure (counts only)."""
    nc = bacc.Bacc("TRN2", num_devices=p.n_cores,
                   dynamic_dma_scratch_size=DMA_SCRATCH,
                   num_swdge_queues=4)
    WPC, SLO = p.WPC, p.SLO
    L1 = p.layers[0]
    L2 = p.layer2
    totK1 = int(L1.K.sum())
    C2 = L2.C2
    n_idx1 = sum(int(L1.CA[s] + L1.CB[s]) for s in range(WPC)) * 128
    n_idx2 = C2 * 128

    # inputs (per core)
    xp_lo = nc.dram_tensor("xp_lo", [p.TLO, D], fp16, kind="ExternalInput")
    xp_hi = nc.dram_tensor("xp_hi", [p.NT - p.TLO, D], fp16, kind="ExternalInput")
    idx1_d = nc.dram_tensor("idx1", [128, n_idx1 // 16], i16, kind="ExternalInput")
    idx2_d = nc.dram_tensor("idx2", [128, n_idx2 // 16], i16, kind="ExternalInput")
    d1_d = nc.dram_tensor("d1", [128, totK1], fp16, kind="ExternalInput")
    gid_d = nc.dram_tensor("gid2", [128, C2], fp16, kind="ExternalInput")
    dvd_d = nc.dram_tensor("dvd2", [128, C2], fp16, kind="ExternalInput")
    NG = p.n_graphs
    dinv_d = nc.dram_tensor("dinvs", [128, WPC], f32, kind="ExternalInput")
    w1_d = nc.dram_tensor("w1", [D, D], fp16, kind="ExternalInput")
    w2_d = nc.dram_tensor("w2", [D, D], fp16, kind="ExternalInput")
    gb_d = nc.dram_tensor("gammab", [128, D], f32, kind="ExternalInput")
    bb_d = nc.dram_tensor("betab", [128, D], f32, kind="ExternalInput")
    b1_d = nc.dram_tensor("b1cb", [128, D], f32, kind="ExternalInput")
    iota_d = nc.dram_tensor("iota", [128, 128], fp16, kind="ExternalInput")
    out_d = nc.dram_tensor("g_part", [NG, D], f32, kind="ExternalOutput")

    # internal DRAM
    h1own = nc.dram_tensor("h1own", [p.NPC, D], fp16, kind="Internal")

    with tile.TileContext(nc) as tc:
        with (
            tc.tile_pool(name="const", bufs=1) as cpool,
            tc.tile_pool(name="msg", bufs=2) as mpool,
            tc.tile_pool(name="st", bufs=2) as spool,
            tc.tile_pool(name="work", bufs=3) as wpool,
            tc.tile_pool(name="psA", bufs=2, space="PSUM") as psA,
            tc.tile_pool(name="psZ", bufs=2, space="PSUM") as psZ,
            tc.tile_pool(name="psG", bufs=2, space="PSUM") as psG,
        ):
            def load_const(dram, shape, dtype, tag):
                t = cpool.tile(shape, dtype, tag=tag)
                nc.sync.dma_start(t[:], dram[:])
                return t

            idx1 = load_const(idx1_d, [128, n_idx1 // 16], i16, "idx1")
            idx2 = load_const(idx2_d, [128, n_idx2 // 16], i16, "idx2")
            d1 = load_const(d1_d, [128, totK1], fp16, "d1")
            gid2 = load_const(gid_d, [128, C2], fp16, "gid2")
            dvd2 = load_const(dvd_d, [128, C2], fp16, "dvd2")
            dinvs = load_const(dinv_d, [128, WPC], f32, "dinvs")
            w1 = load_const(w1_d, [D, D], fp16, "w1")
            w2 = load_const(w2_d, [D, D], fp16, "w2")
            gb = load_const(gb_d, [128, D], f32, "gb")
            bb = load_const(bb_d, [128, D], f32, "bb")
            b1cb = load_const(b1_d, [128, D], f32, "b1cb")
            iota = load_const(iota_d, [128, 128], fp16, "iota")
            eps_col = cpool.tile([128, 1], f32, tag="eps")
            nc.vector.memset(eps_col[:], EPS)

            def do_layer(layer_i, idx_sb, d_sb, tab_lo, tab_hi, w_sb):
                do_layer.q = getattr(do_layer, "q", 0)
                Lp = p.layers[layer_i]
                icol = 0  # running idx column (16 idx per col)
                dcol = 0  # running d/msg chunk column
                for b in Lp.batches:
                    nA = sum(int(Lp.CA[s]) for s in b)
                    nB = sum(int(Lp.CB[s]) for s in b)
                    kc = nA + nB
                    MC = Lp.MSGC
                    msg = mpool.tile([128, MC, 128], fp16, tag="msg")
                    st = spool.tile([128, MC, 128], fp16, tag="st")
                    for grp, n_ch, tab in ((0, nA, tab_lo), (1, nB, tab_hi)):
                        off = 0 if grp == 0 else nA
                        for k0 in range(0, n_ch, CAPC):
                            kn = min(CAPC, n_ch - k0)
                            ni = kn * 128
                            nc.gpsimd.dma_gather(
                                msg[:, off + k0:off + k0 + kn, :],
                                tab[:, :],
                                idx_sb[:, icol:icol + ni // 16],
                                ni, ni, D,
                                queue_num=do_layer.q % 4,
                            )
                            do_layer.q += 1
                            icol += ni // 16
                    # one-hot dst indicator for the whole batch
                    dsl = d_sb[:, dcol:dcol + kc]
                    nc.vector.tensor_tensor(
                        out=st[:, 0:kc, :],
                        in0=dsl.unsqueeze(2).broadcast_to([128, kc, 128]),
                        in1=iota[:].unsqueeze(1).broadcast_to([128, kc, 128]),
                        op=Alu.is_equal,
                    )
                    bcol = 0  # col offset within batch, A region then B region
                    aoff, boff = 0, nA
                    for s in b:
                        ka, kb = int(Lp.CA[s]), int(Lp.CB[s])
                        cols = list(range(aoff, aoff + ka)) + \
                               list(range(boff, boff + kb))
                        aoff += ka
                        boff += kb
                        kk = ka + kb
                        if kk == 0:
                            continue
                        agg = psA.tile([128, 128], f32, tag="agg")
                        for j, col in enumerate(cols):
                            nc.tensor.matmul(
                                agg[:],
                                lhsT=msg[:, col, :],
                                rhs=st[:, col, :],
                                start=(j == 0),
                                stop=(j == kk - 1),
                            )
                        aggs = wpool.tile([128, 128], fp16, tag="aggs")
                        nc.scalar.copy(aggs[:], agg[:])
                        zp = psZ.tile([128, 128], f32, tag="z")
                        dv = dinvs[:, s:s + 1]
                        if True:
                            nc.tensor.matmul(zp[:], lhsT=aggs[:], rhs=w_sb[:],
                                             start=True, stop=True)
                            # z*dinv[dst] + b1 -> LayerNorm -> ReLU, then
                            # *dinv again as the next layer's table prescale
                            zb = wpool.tile([128, 128], f32, tag="zb")
                            zsum = wpool.tile([128, 1], f32, tag="zsum")
                            nc.scalar.activation(zb[:], zp[:], Act.Identity,
                                                 scale=dv, accum_out=zsum[:])
                            mu = wpool.tile([128, 1], f32, tag="mu")
                            nc.scalar.mul(mu[:], zsum[:], 1.0 / D)
                            t0 = wpool.tile([128, 128], f32, tag="t0")
                            nc.vector.tensor_scalar_sub(t0[:], zb[:], mu[:])
                            t = wpool.tile([128, 128], f32, tag="t")
                            nc.vector.tensor_add(t[:], t0[:], b1cb[:])
                            tsq = wpool.tile([128, 128], f32, tag="tsq")
                            vsum = wpool.tile([128, 1], f32, tag="vsum")
                            nc.scalar.activation(tsq[:], t[:], Act.Square,
                                                 accum_out=vsum[:])
                            std = wpool.tile([128, 1], f32, tag="std")
                            nc.scalar.activation(std[:], vsum[:], Act.Sqrt,
                                                 bias=eps_col[:], scale=1.0 / D)
                            rstd = wpool.tile([128, 1], f32, tag="rstd")
                            nc.vector.reciprocal(rstd[:], std[:])
                            tg = wpool.tile([128, 128], f32, tag="tg")
                            nc.vector.tensor_scalar_mul(tg[:], t[:], rstd[:])
                            y = wpool.tile([128, 128], f32, tag="y")
                            nc.vector.tensor_mul(y[:], tg[:], gb[:])
                            y2 = wpool.tile([128, 128], f32, tag="y2")
                            nc.vector.tensor_add(y2[:], y[:], bb[:])
                            h1q = wpool.tile([128, 128], fp16, tag="h1q")
                            nc.scalar.activation(h1q[:], y2[:], Act.Relu,
                                                 scale=dv)
                            nc.sync.dma_start(
                                h1own[s * 128:(s + 1) * 128, :], h1q[:])
                    dcol += kc

            do_layer(0, idx1, d1, xp_lo, xp_hi, w1)

            # ---- layer 2: src-sharded pooled partial R = sum_e msg_e (x) U_e
            # U[e, g] = (batch[dst_e] == g) * dinv[dst_e]; R accumulates over
            # every chunk in one [128, NG] PSUM tile; g_part = R.T @ W2.
            Rp = psG.tile([128, NG], f32, tag="R")
            nchunks = C2
            first = True
            icol2 = 0
            for (k0, k1) in L2.batches:
                kc = k1 - k0
                msg = mpool.tile([128, L2.batches[0][1], 128], fp16, tag="msg")
                u0 = spool.tile([128, L2.batches[0][1], NG], fp16, tag="u0")
                uu = spool.tile([128, L2.batches[0][1], NG], fp16, tag="uu")
                for c0 in range(0, kc, CAPC):
                    cn = min(CAPC, kc - c0)
                    ni = cn * 128
                    nc.gpsimd.dma_gather(
                        msg[:, c0:c0 + cn, :],
                        h1own[:, :],
                        idx2[:, icol2:icol2 + ni // 16],
                        ni, ni, D,
                        queue_num=do_layer.q % 4,
                    )
                    do_layer.q += 1
                    icol2 += ni // 16
                gsl = gid2[:, k0:k1]
                dsl = dvd2[:, k0:k1]
                nc.vector.tensor_tensor(
                    u0[:, 0:kc, :],
                    gsl.unsqueeze(2).broadcast_to([128, kc, NG]),
                    iota[:, 0:NG].unsqueeze(1).broadcast_to([128, kc, NG]),
                    Alu.is_equal,
                )
                nc.vector.tensor_tensor(
                    uu[:, 0:kc, :],
                    u0[:, 0:kc, :],
                    dsl.unsqueeze(2).broadcast_to([128, kc, NG]),
                    Alu.mult,
                )
                for j in range(kc):
                    nc.tensor.matmul(
                        Rp[:],
                        lhsT=msg[:, j, :],
                        rhs=uu[:, j, :],
                        start=first,
                        stop=(k1 == nchunks and j == kc - 1),
                    )
                    first = False
            Rs = wpool.tile([128, NG], fp16, tag="Rs")
            nc.scalar.copy(Rs[:], Rp[:])
            gps = psZ.tile([NG, D], f32, tag="gps")
            nc.tensor.matmul(gps[:], lhsT=Rs[:], rhs=w2[:],
                             start=True, stop=True)
            gout = wpool.tile([NG, D], f32, tag="gout")
            nc.scalar.copy(gout[:], gps[:])
            nc.sync.dma_start(out_d[:, :], gout[:])

    nc.compile()
    return nc


def _prep_inputs(p, x, batch, W1, b1, gamma, beta, W2):
    """Host-side tensor prep shared across cores + per-core arrays."""
    n_nodes, WPC = p.n_nodes, p.WPC
    xp = np.zeros((p.NT, D), dtype=np.float16)
    xp[:n_nodes] = (x * p.dinv[:, None]).astype(np.float16)
    com = {
        "xp_lo": xp[:p.TLO],
        "xp_hi": xp[p.TLO:],
        "w1": np.ascontiguousarray(W1.astype(np.float16)),
        "w2": np.ascontiguousarray(W2.astype(np.float16)),
        "gammab": np.ascontiguousarray(
            np.broadcast_to(gamma.astype(np.float32), (128, D))),
        "betab": np.ascontiguousarray(
            np.broadcast_to(beta.astype(np.float32), (128, D))),
        "b1cb": np.ascontiguousarray(np.broadcast_to(
            (b1 - b1.mean()).astype(np.float32), (128, D))),
        "iota": np.ascontiguousarray(
            np.broadcast_to(np.arange(128, dtype=np.float16), (128, 128))),
    }
    in_maps = []
    for c in range(p.n_cores):
        idx1, d1 = _core_data(p, 0, c)
        idx2, gid2, dvd2 = _core_data2(p, c)
        # per-slot dinv columns (layer-1 epilogue scale)
        dinvs = np.ones((128, WPC), dtype=np.float32)
        for s in range(WPC):
            w = int(p.slotwins[c][s])
            v0 = w * 128
            n = min(128, n_nodes - v0)
            if n <= 0:
                continue
            dinvs[:n, s] = p.dinv[v0:v0 + n]
        m = dict(com)
        m.update({"idx1": idx1, "idx2": idx2, "d1": d1,
                  "gid2": gid2, "dvd2": dvd2, "dinvs": dinvs})
        in_maps.append(m)
    return in_maps


def run_gcn(x, src, dst, batch, W1, b1, gamma, beta, W2, b2,
            n_nodes, n_graphs, n_cores=N_CORES, trace=False):
    x = np.asarray(x, dtype=np.float32)
    src = np.asarray(src).astype(np.int64)
    dst = np.asarray(dst).astype(np.int64)
    batch = np.asarray(batch).astype(np.int64)
    p = _plan(src, dst, batch, n_nodes, n_cores, n_graphs)
    nc = _build_nc(p)
    in_maps = _prep_inputs(p, x, batch, np.asarray(W1), np.asarray(b1),
                           np.asarray(gamma), np.asarray(beta), np.asarray(W2))
    res = run_bass_kernel_spmd(nc, in_maps, list(range(n_cores)), trace=trace)
    run_gcn.last_nc = nc
    run_gcn.last_plan = p
    gsum = np.sum([r["g_part"] for r in res.results], axis=0)
    counts = np.bincount(batch, minlength=n_graphs).astype(np.float32)
    g = gsum / np.maximum(counts, 1.0)[:, None] + np.asarray(b2, np.float32)
    # reference gives 0 + b2*0 contribution for empty graphs? No: reference
    # pools zeros then adds nothing (b2 enters per-node, pre-pool) -> 0.
    g[counts == 0] = 0.0
    return g.astype(np.float32), res


def kernel(x, src, dst, batch, W1, b1, gamma, beta, W2, b2):
    g, _ = run_gcn(x, src, dst, batch, W1, b1, gamma, beta, W2, b2,
                   N_NODES, N_GRAPHS)
    return g
